# revision 1
# baseline (speedup 1.0000x reference)
"""Trainium2 Bass kernel for nn_Conv2DLayer_16011638080159.

Math: out = C * (x @ weight.sum(0))   with x [524288, 512], weight [9, 512].
Equivalent to a row-wise dot product of x with w_eff = C * weight.sum(0).

Strategy (pure data parallel, per sharding hint):
  - Shard x along the batch axis across 8 NeuronCores (65536 rows each).
  - Host-side prep: fold the tiny K=9 weight sum and the C scale into a
    single [C] vector, replicated to a [128, 8*C] SBUF-ready constant.
  - Per core: stream x in [128 partitions, 8 rows x 512] tiles from HBM
    with 6-deep buffering, alternating the two HWDGE rings. The kernel is
    HBM bound (~415 us/core pure-DMA floor measured at 8 cores), so the
    row-dot-products are split so each compute engine stays below that:
      * Vector engine: fp32 tensor_tensor multiply of the whole tile by
        the replicated weight (1x mode), plus a segmented tensor_reduce
        for 1 of the 8 rows  (~320 us/core busy).
      * Scalar engine: the other 7 rows via ACTIVATE(Copy, accum_out),
        which sums 512 elems/row at 1 elem/cycle (~355 us/core busy).
  - Row mapping: shard row (p*512 + t*R + r) sits at partition p, tile t,
    slot r, so the per-core result tile [128, 512] is exactly the row-major
    view of the per-core output [65536]; one contiguous DMA writes it out.
"""

import numpy as np

import concourse.bacc as bacc
import concourse.bass as bass
import concourse.tile as tile
from concourse import mybir
from concourse.bass_utils import run_bass_kernel_spmd

B = 524288        # total rows
C = 512           # row length
N_CORES = 8
BS = B // N_CORES  # 65536 rows per core
P = 128            # SBUF partitions
RPP = BS // P      # 512 rows per partition
R = 8              # rows per partition per tile
F = R * C          # 4096 free elems per tile
NT = RPP // R      # 64 tiles per core
K_DVE = 1          # rows per tile reduced on DVE via segmented tensor_reduce

_NC_CACHE = None
LAST_RESULT = None  # BassKernelResults of the most recent run (for profiling)


def _build() -> bass.Bass:
    # Bacc (not raw Bass): its compile() pass splits multi-sem waits into
    # EventSemaphore instructions -- the TRN2 ISA allows only 1 wait/inst.
    nc = bacc.Bacc(None, target_bir_lowering=False, debug=False)
    x = nc.dram_tensor("x", [BS, C], mybir.dt.float32, kind="ExternalInput")
    w = nc.dram_tensor("w", [P, F], mybir.dt.float32, kind="ExternalInput")
    out = nc.dram_tensor("out", [BS], mybir.dt.float32, kind="ExternalOutput")

    # shard row (p*RPP + t*R + r) -> partition p, tile t, free slot (r, c)
    xv = x.rearrange("(p t r) c -> t p (r c)", p=P, t=NT, r=R)
    ov = out.rearrange("(p f) -> p f", p=P)

    n_act = R - K_DVE  # rows per tile reduced on the Scalar engine

    with tile.TileContext(nc) as tc:
        with (
            tc.tile_pool(name="const", bufs=1) as cpool,
            tc.tile_pool(name="xs", bufs=6) as xs,
            tc.tile_pool(name="ys", bufs=4) as ys,
            tc.tile_pool(name="scr", bufs=2) as scr,
            tc.tile_pool(name="res", bufs=1) as res,
        ):
            w_t = cpool.tile([P, F], mybir.dt.float32)
            nc.sync.dma_start(out=w_t[:], in_=w[:, :])
            o_t = res.tile([P, RPP], mybir.dt.float32)
            for t in range(NT):
                # All x DMAs go on the SP HWDGE ring: SP has no compute, so
                # DMA issue is never queued behind engine work (issuing from
                # nc.scalar stalls the DMA behind pending ACTIVATEs).
                x_t = xs.tile([P, F], mybir.dt.float32)
                nc.sync.dma_start(out=x_t[:], in_=xv[t])

                # one fp32 TT multiply for the whole tile
                y_t = ys.tile([P, F], mybir.dt.float32)
                nc.vector.tensor_mul(y_t[:], x_t[:], w_t[:])

                # ACT accumulates rows K_DVE..R-1 (one 512-sum per row)
                for r in range(n_act):
                    s_t = scr.tile([P, C], mybir.dt.float32, tag="act_s")
                    col = t * R + K_DVE + r
                    nc.scalar.activation(
                        out=s_t[:],
                        in_=y_t[:, (K_DVE + r) * C:(K_DVE + r + 1) * C],
                        func=mybir.ActivationFunctionType.Copy,
                        accum_out=o_t[:, col: col + 1],
                    )

                # DVE reduces rows 0..K_DVE-1 in one segmented reduce
                nc.vector.tensor_reduce(
                    out=o_t[:, t * R: t * R + K_DVE],
                    in_=y_t[:, 0:K_DVE * C].rearrange("p (r c) -> p r c", c=C),
                    axis=mybir.AxisListType.X,
                    op=mybir.AluOpType.add,
                )
            nc.sync.dma_start(out=ov, in_=o_t[:])
    nc.finalize()
    return nc


def kernel(x: np.ndarray, weight: np.ndarray) -> np.ndarray:
    global _NC_CACHE, LAST_RESULT
    x = np.ascontiguousarray(np.asarray(x), dtype=np.float32)
    weight = np.asarray(weight, dtype=np.float32)

    w_eff = (C * weight.sum(axis=0)).astype(np.float32)   # [C]
    w_rep = np.ascontiguousarray(np.tile(w_eff, (P, R)))  # [P, F]

    if _NC_CACHE is None:
        _NC_CACHE = _build()

    in_maps = [
        {"x": x[i * BS:(i + 1) * BS], "w": w_rep} for i in range(N_CORES)
    ]
    LAST_RESULT = run_bass_kernel_spmd(
        _NC_CACHE, in_maps, core_ids=list(range(N_CORES))
    )
    return np.concatenate([r["out"] for r in LAST_RESULT.results])



# revision 6
# speedup vs baseline: 2.1519x; 2.1519x over previous
"""Trainium2 Bass kernel for nn_Conv2DLayer_16011638080159.

Math: out = C * (x @ weight.sum(0))   with x [524288, 512], weight [9, 512].
A row-wise dot product of x with w_eff = C * weight.sum(0).

Strategy (v3, PE-based, fp16 traffic):
  - Pure data parallel: shard x rows across 8 NeuronCores (65536 rows each).
  - The kernel is HBM-DMA bound, so HBM traffic is halved by casting x to
    fp16 on the host (l2 rel err 2.5e-4, gate is 2e-2). The host also
    pre-transposes each shard to x^T [512, 65536] so the TensorEngine can
    reduce along the channel (partition) axis with plain contiguous DMAs.
  - Per core: stream x^T in [128, 8192] fp16 tiles (2 MB DMAs; 4 channel
    chunks x 8 row-blocks) on the SP HWDGE ring. For each group of 512
    rows: 4 PSUM-accumulating matmuls, stationary = w chunk [128, 1],
    moving = x^T tile slice [128, 512], out = psum [1, 512].
  - PSUM partition-0 rows are copied (cast) to an fp16 result row
    o16[1, 512*g..] alternating DVE/ACT to split the load. w is scaled by
    1/16 so the fp16 partial results stay far below the fp16 max; the
    host multiplies the gathered output by 16 in fp32.
  - Engine budgets/core: DMA ~192 us (bound), PE ~123 us, DVE/ACT ~40 us.
"""

import numpy as np

import concourse.bacc as bacc
import concourse.bass as bass
import concourse.tile as tile
from concourse import mybir
from concourse.bass_utils import run_bass_kernel_spmd

B = 524288         # total rows
C = 512            # row length (channels)
N_CORES = 8
BS = B // N_CORES  # 65536 rows per core
P = 128            # SBUF/PSUM partitions
KC = C // P        # 4 channel chunks of 128
RB = 8192          # rows per super-block (x tile free dim)
NT = BS // RB      # 8 super-blocks per core
GPB = RB // 512    # 16 groups of 512 rows per super-block
OUT_SCALE = 16.0   # host multiplies fp16 device output by this (w /= 16)

_NC_CACHE = None
LAST_RESULT = None  # BassKernelResults of the most recent run (for profiling)


def _build() -> bass.Bass:
    nc = bacc.Bacc(None, target_bir_lowering=False, debug=False)
    xT = nc.dram_tensor("xT", [C, BS], mybir.dt.float16, kind="ExternalInput")
    w = nc.dram_tensor("w", [P, KC], mybir.dt.float16, kind="ExternalInput")
    out = nc.dram_tensor("out", [BS], mybir.dt.float16, kind="ExternalOutput")

    # chunk k, super-block t -> [128 partitions, RB rows], contiguous lines
    xv = xT.rearrange("(k p) (t f) -> k t p f", k=KC, p=P, t=NT, f=RB)
    ov = out.rearrange("(o f) -> o f", o=1)

    with tile.TileContext(nc) as tc:
        with (
            tc.tile_pool(name="const", bufs=1) as cpool,
            tc.tile_pool(name="xs", bufs=2) as xs,
            tc.psum_pool(name="ps", bufs=4) as ps,
            tc.tile_pool(name="res", bufs=2) as res,
        ):
            w_t = cpool.tile([P, KC], mybir.dt.float16)
            nc.sync.dma_start(out=w_t[:], in_=w[:, :])
            for t in range(NT):
                xk = []
                for k in range(KC):
                    x_t = xs.tile([P, RB], mybir.dt.float16, tag=f"x{k}")
                    nc.sync.dma_start(out=x_t[:], in_=xv[k, t])
                    xk.append(x_t)
                o_t = res.tile([1, RB], mybir.dt.float16)
                for j in range(GPB):
                    g = t * GPB + j
                    p_t = ps.tile([1, 512], mybir.dt.float32)
                    for k in range(KC):
                        nc.tensor.matmul(
                            p_t[:],
                            lhsT=w_t[:, k:k + 1],
                            rhs=xk[k][:, j * 512:(j + 1) * 512],
                            start=(k == 0),
                            stop=(k == KC - 1),
                        )
                    dst = o_t[:, j * 512:(j + 1) * 512]
                    if g % 2 == 0:
                        nc.vector.tensor_copy(dst, p_t[:])
                    else:
                        nc.scalar.copy(dst, p_t[:])
                # off the SP ring so x-tile DMA issue is never delayed
                nc.scalar.dma_start(out=ov[:, t * RB:(t + 1) * RB], in_=o_t[:])
    nc.finalize()
    return nc


def kernel(x: np.ndarray, weight: np.ndarray) -> np.ndarray:
    global _NC_CACHE, LAST_RESULT
    x = np.asarray(x)
    weight = np.asarray(weight, dtype=np.float32)

    w_eff = (C / OUT_SCALE * weight.sum(axis=0)).astype(np.float16)  # [C]
    w_stat = np.ascontiguousarray(w_eff.reshape(KC, P).T)            # [P, KC]

    # fp16 cast (contiguous pass), then per-shard transpose to [C, BS]
    x16 = np.asarray(x, dtype=np.float16)
    shards = [
        np.ascontiguousarray(x16[i * BS:(i + 1) * BS].T) for i in range(N_CORES)
    ]

    if _NC_CACHE is None:
        _NC_CACHE = _build()

    in_maps = [{"xT": shards[i], "w": w_stat} for i in range(N_CORES)]
    LAST_RESULT = run_bass_kernel_spmd(
        _NC_CACHE, in_maps, core_ids=list(range(N_CORES))
    )
    return np.concatenate(
        [r["out"].astype(np.float32) * OUT_SCALE for r in LAST_RESULT.results]
    )
